# revision 22
# baseline (speedup 1.0000x reference)
"""Trainium2 Bass kernel for nn_Fractal1D (soft fractal / smoothed decision-tree descent).

Reference computation (per point x, N=131072 points, M=128 nodes, depth 10):
    split = sigmoid(4*p - 2); values = tile(3*v + 1, 4)
    w0 = e_0;  lo=0, hi=1
    repeat 10x:
        s  = lo + (w @ split) * (hi - lo)
        t  = sigmoid((x - s) / 0.1)
        w  = (1-t) * (w @ L) + t * (w @ R)
        lo, hi = (1-t)*lo + t*s, (1-t)*s + t*hi
    out = w @ values

Kernel strategy (data-parallel over 8 cores, 16384 points/core):
  * w^T resident in SBUF as [128 nodes, 16384 points] in bf16; points processed
    in 32 chunks of 512 (one PSUM bank of fp32 per matmul output).
  * All matmul operands are bf16: weight loads use FWL and the PE streams
    multiple columns/cycle (~60ns per [128x128]@[128x512] matmul measured, vs
    ~390ns for fp32r with rotating weights). fp32 PSUM accumulation keeps the
    numerics at rel err ~8e-3 (gate is 2e-2).
  * Parameter transforms (sigmoid(4p-2), 3v+1 tile, one-hot placed splitE and
    esel broadcast masks, pushed-through value tables) are tiny and
    precomputed on host in make_in_maps.
  * Per depth the chunk loop is software-pipelined two chunks deep so the
    in-order PE stream never makes the DVE wait on a cross-engine round trip:
      stage 1 (chunk c):  tb_c = broadcast matmul (lhsT = one-hot esel row)
                          replicating chunk c's t row across 128 partitions
                          into PSUM; L matmul into a half of the paired wn
                          tile; vv_c = w_c * tb_c (one DVE op, bf16 out).
      stage 2 (chunk c-2): (R-L)^T vv matmul into the other accumulation
                          group half; one paired ACT copy [128,1024] moves
                          wn PSUM -> bf16 SBUF per two chunks.
  * sdot matvec: 16 accumulating matmuls per phase with one-hot-placed split
    columns stack 16 chunks into one [16, 512] PSUM region (lagged behind the
    copies).
  * Depth 0 is closed-form: w1 = t_b*(R-L)[0,:] + L[0,:] written directly
    from the broadcast (alternating DVE tensor_scalar / ACT Relu, exact
    since w1 >= 0).
  * The last TWO depths are fused into the output ("fold-2"): for any
    constant u, w_9 . u = w_8 . (L@u) + t_8 * (w_8 . ((R-L)@u)), so sdot_9
    and both output dots reduce to 6 matvec variants on w_8 (one-hot packed
    4 + 2 per PSUM bank at partition offsets 0/32/64/96 via the f1E/f2E
    tables, 2 matmuls per chunk) plus a short per-point combine:
        sdot9 = dLs + t8*dRms;  t9 = sigmoid(10*(xml9 - sdot9*dd9))
        y = (dA1 + t8*dA2) + t9*(dB1 + t8*dB2)
    w_9 is never materialized: no broadcasts, vv, or copies for depths 8-9.
  * Row math runs 16-chunks-wide on partitions 0..15 with the substitution
    xml = x - lo, d = hi - lo:
        g = sdot * d; t = sigmoid(10*(xml - g)); xml' = xml - t*g
        d' = g - 2*t*g + t*d
    Phase k runs group k%2 at depth k//2; row math for phase k+1 is emitted
    early in phase k (chunk 2) so its DVE->GPSIMD->ACT chain hides under
    phase k's chunk work.
  * PSUM budget: tb (2 banks) + paired wn (4 banks) + sdot/yab (2 banks).
"""

from contextlib import ExitStack

import ml_dtypes
import numpy as np

import concourse.bacc as bacc
import concourse.tile as tile
from concourse import mybir
from concourse.bass_utils import run_bass_kernel_spmd

F32 = mybir.dt.float32
BF16 = mybir.dt.bfloat16
NP_BF16 = ml_dtypes.bfloat16
AOP = mybir.AluOpType
AFT = mybir.ActivationFunctionType

N_TOTAL = 131072
NCORES = 8
NPTS = N_TOTAL // NCORES      # 16384 points per core
F = 512                       # points per chunk (one PSUM bank of fp32)
M = 128                       # fractal nodes
NCH = 32                      # chunks (= partitions used for batched row math)
DEPTH = 10
INV_SMOOTH = 10.0             # 1 / smoothing_width
G = NCH // 2                  # 16 chunks per phase group
MV_LAG = 4                    # chunks of lag before issuing sdot matvecs


def _emit(nc, bench_reps=1):
    x_in = nc.declare_dram_parameter("x", [NPTS], F32, isOutput=False)
    l16_in = nc.declare_dram_parameter("l16", [M, M], BF16, isOutput=False)
    rml16_in = nc.declare_dram_parameter("rml16", [M, M], BF16, isOutput=False)
    splitE_in = nc.declare_dram_parameter("splitE", [M, G * G], BF16, isOutput=False)
    f1E_in = nc.declare_dram_parameter("f1E", [M, G * M], BF16, isOutput=False)
    f2E_in = nc.declare_dram_parameter("f2E", [M, G * M], BF16, isOutput=False)
    esel_in = nc.declare_dram_parameter("esel", [G, G * M], BF16, isOutput=False)
    l0col_in = nc.declare_dram_parameter("l0col", [M, 1], F32, isOutput=False)
    rml0_in = nc.declare_dram_parameter("rml0", [M, 1], F32, isOutput=False)
    sp0_in = nc.declare_dram_parameter("sp0", [G, 1], F32, isOutput=False)
    b0_in = nc.declare_dram_parameter("b0", [G, 1], F32, isOutput=False)
    y_out = nc.declare_dram_parameter("y", [NPTS], F32, isOutput=True)

    with tile.TileContext(nc) as tc, ExitStack() as ctx:
        sing = ctx.enter_context(tc.tile_pool(name="sing", bufs=1))
        scratch = ctx.enter_context(tc.tile_pool(name="scratch", bufs=2))
        tpool = ctx.enter_context(tc.tile_pool(name="tpool", bufs=4))
        vpool = ctx.enter_context(tc.tile_pool(name="vpool", bufs=3))
        ps_t = ctx.enter_context(tc.tile_pool(name="ps_t", bufs=2, space="PSUM"))
        ps_w = ctx.enter_context(tc.tile_pool(name="ps_w", bufs=2, space="PSUM"))
        ps_sdot = ctx.enter_context(tc.tile_pool(name="ps_sdot", bufs=2, space="PSUM"))

        # ---- constants (host-precomputed, DMA'd once) ----
        def load(name, shape, dt, src):
            t = sing.tile(shape, dt, tag=name)
            nc.sync.dma_start(out=t, in_=src)
            return t

        l16 = load("l16", [M, M], BF16, l16_in[:, :])
        rml16 = load("rml16", [M, M], BF16, rml16_in[:, :])
        splitE = load("splitE", [M, G * G], BF16, splitE_in[:, :])
        f1E = load("f1E", [M, G * M], BF16, f1E_in[:, :])
        f2E = load("f2E", [M, G * M], BF16, f2E_in[:, :])
        esel = load("esel", [G, G * M], BF16, esel_in[:, :])
        l0col = load("l0col", [M, 1], F32, l0col_in[:, :])
        rml0 = load("rml0", [M, 1], F32, rml0_in[:, :])
        sp0 = load("sp0", [G, 1], F32, sp0_in[:, :])
        b0 = load("b0", [G, 1], F32, b0_in[:, :])

        def esel_slice(i):
            return esel[:, i * M : (i + 1) * M]

        # ---- resident state (per group) ----
        w_bufs = [
            sing.tile([M, NPTS], BF16, tag="w_ping", name="w_ping"),
            sing.tile([M, NPTS], BF16, tag="w_pong", name="w_pong"),
        ]
        half = G * F
        xml = [
            sing.tile([G, F], F32, tag="xml0", name="xml0"),
            sing.tile([G, F], F32, tag="xml1", name="xml1"),
        ]
        dd = [
            sing.tile([G, F], F32, tag="dd0", name="dd0"),
            sing.tile([G, F], F32, tag="dd1", name="dd1"),
        ]

        def body():
         for g in range(2):
            nc.sync.dma_start(
                out=xml[g],
                in_=x_in[g * half : (g + 1) * half].rearrange("(p f) -> p f", f=F),
            )

         # ---- depth 0: closed form state; w1 via fused outer-product blend ----
         ttile = [None, None]
         for g in range(2):
             t0 = tpool.tile([G, F], BF16, tag="t", name=f"t0g{g}")
             nc.scalar.activation(t0, xml[g], AFT.Sigmoid, bias=b0, scale=INV_SMOOTH)
             tgt = scratch.tile([G, F], F32, tag="tgt")
             nc.gpsimd.tensor_scalar_mul(tgt, t0, sp0)
             nc.gpsimd.tensor_sub(xml[g], xml[g], tgt)
             ee = scratch.tile([G, F], F32, tag="ee")
             nc.gpsimd.tensor_scalar_mul(ee, tgt, -2.0)
             nc.gpsimd.tensor_add(ee, ee, t0)
             nc.gpsimd.tensor_scalar(dd[g], ee, sp0, None, op0=AOP.add)
             ttile[g] = t0

         # Phase schedule: phases k = 0..2*DEPTH-1 map to (depth, group).
         # Phase k runs group k%2 at depth k//2. The matvec feeding phase k+2
         # is accumulated (lagged) inside phase k's pair loop; the row math
         # for phase k+1 is emitted before phase k's pairs so its short
         # DVE->ACT chain hides under phase k's PE work.
         sdot_ps = [None] * (2 * DEPTH + 2)

         def row_math(k):
             """State update for phase k (depth k//2 >= 1, group k%2)."""
             g = k % 2
             sdot = sdot_ps[k]
             gt = scratch.tile([G, F], F32, tag="gt", name=f"gt{k}")
             nc.vector.tensor_mul(gt, sdot[0:G, :], dd[g])
             xms = scratch.tile([G, F], F32, tag="xms", name=f"xms{k}")
             nc.gpsimd.tensor_sub(xms, xml[g], gt)
             tg_t = tpool.tile([G, F], BF16, tag="t", name=f"t{k}")
             nc.scalar.activation(tg_t, xms, AFT.Sigmoid, scale=INV_SMOOTH)
             if k // 2 < DEPTH - 1:
                 # xml/dd not needed after the last depth's t
                 tgt = scratch.tile([G, F], F32, tag="tgt", name=f"tgt{k}")
                 nc.gpsimd.tensor_mul(tgt, tg_t, gt)
                 nc.gpsimd.tensor_sub(xml[g], xml[g], tgt)
                 ee = scratch.tile([G, F], F32, tag="ee", name=f"ee{k}")
                 nc.gpsimd.tensor_scalar_mul(ee, tgt, -2.0)
                 nc.gpsimd.tensor_add(ee, ee, gt)
                 td = scratch.tile([G, F], F32, tag="td", name=f"td{k}")
                 nc.gpsimd.tensor_mul(td, tg_t, dd[g])
                 nc.gpsimd.tensor_add(dd[g], td, ee)
             ttile[g] = tg_t

         for k in range(2 * DEPTH - 4):
             dep, g = k // 2, k % 2
             w_cur = w_bufs[dep % 2]
             w_new = w_bufs[(dep + 1) % 2]
             tg_t = ttile[g]
             final = dep == DEPTH - 1
             if not final:
                 sdot_next = ps_sdot.tile([M, F], F32, tag="sdot", name=f"sdot{k + 2}")
                 sdot_ps[k + 2] = sdot_next

             mv_q = []

             def flush_mv(limit):
                 while len(mv_q) > limit:
                     j = mv_q.pop(0)
                     cj = g * G + j
                     nc.tensor.matmul(
                         sdot_next[0:G, :],
                         lhsT=splitE[:, j * G : (j + 1) * G],
                         rhs=w_new[:, cj * F : (cj + 1) * F],
                         start=(j == 0),
                         stop=(j == G - 1),
                     )

             # Software-pipelined chunk loop: stage 1 (tb broadcast, L matmul,
             # vv) runs two chunks ahead of stage 2 (RmL matmul, copy, lagged
             # matvec) so the in-order PE stream never makes the DVE wait for
             # a cross-engine round trip.
             if final:
                 # t factors out of the value dots: y = (L@v)^T w_9 +
                 # t_9 * ((R-L)@v)^T w_9. One matmul per chunk accumulates
                 # both dot sets into a single bank (A rows 0..15, B rows
                 # 32..47 via one-hot column placement in yE).
                 yab = ps_sdot.tile([M, F], F32, tag="sdot", name=f"yab{g}")
                 for c in range(G):
                     ci = g * G + c
                     nc.tensor.matmul(
                         yab,
                         lhsT=yE[:, c * M : (c + 1) * M],
                         rhs=w_cur[:, ci * F : (ci + 1) * F],
                         start=(c == 0),
                         stop=(c == G - 1),
                     )
                     if c == 2 and k + 1 < 2 * DEPTH:
                         row_math(k + 1)
                 tg9 = ttile[g]
                 ym = scratch.tile([G, F], F32, tag="ym", name=f"ym{g}")
                 nc.vector.tensor_mul(ym, tg9, yab[32 : 32 + G, :])
                 ysb = scratch.tile([G, F], F32, tag="ysb", name=f"ysb{g}")
                 nc.vector.tensor_add(ysb, ym, yab[0:G, :])
                 nc.sync.dma_start(
                     out=y_out[g * half : (g + 1) * half].rearrange(
                         "(p f) -> p f", f=F
                     ),
                     in_=ysb,
                 )
                 continue

             tb_t = [None] * G
             vv_t = [None] * G
             wn_t = [None] * (G // 2)
             for c in range(G + 2):
                 if c < G:
                     ci = g * G + c
                     wsl = w_cur[:, ci * F : (ci + 1) * F]
                     tb = ps_t.tile([M, F], F32, tag="tb", name=f"tb{k}_{c}")
                     nc.tensor.matmul(
                         tb, lhsT=esel_slice(c), rhs=tg_t, start=True, stop=True
                     )
                     tb_t[c] = tb
                     if dep == 0:
                         # w1 = t_b*(R-L)[0,:] + L[0,:]; w1 >= 0 so the ACT
                         # Relu form is exact. Alternate engines to balance.
                         if c % 2 == 0:
                             nc.vector.tensor_scalar(
                                 w_new[:, ci * F : (ci + 1) * F], tb, rml0, l0col,
                                 op0=AOP.mult, op1=AOP.add,
                             )
                         else:
                             nc.scalar.activation(
                                 w_new[:, ci * F : (ci + 1) * F], tb, AFT.Relu,
                                 bias=l0col, scale=rml0,
                             )
                         mv_q.append(c)
                         flush_mv(MV_LAG)
                     else:
                         if c % 2 == 0:
                             wn_t[c // 2] = ps_w.tile(
                                 [M, 2 * F], F32, tag="wn2", name=f"wn2_{k}_{c // 2}"
                             )
                         nc.tensor.matmul(
                             wn_t[c // 2][:, (c % 2) * F : (c % 2 + 1) * F],
                             lhsT=l16, rhs=wsl, start=True, stop=False,
                         )
                         vv = vpool.tile([M, F], BF16, tag="vv", name=f"vv{k}_{c}")
                         nc.vector.tensor_mul(vv, wsl, tb)
                         vv_t[c] = vv
                 if c == 2 and 2 <= k + 1 < 2 * DEPTH:
                     row_math(k + 1)
                 d = c - 2
                 if 0 <= d < G and dep >= 1:
                     nc.tensor.matmul(
                         wn_t[d // 2][:, (d % 2) * F : (d % 2 + 1) * F],
                         lhsT=rml16, rhs=vv_t[d], start=False, stop=True,
                     )
                     if d % 2 == 1:
                         pr = d // 2
                         c0 = g * G + 2 * pr
                         nc.scalar.copy(
                             w_new[:, c0 * F : (c0 + 2) * F], wn_t[pr]
                         )
                         mv_q.append(2 * pr)
                         mv_q.append(2 * pr + 1)
                         flush_mv(MV_LAG)
             flush_mv(0)

         # ---- fold phases: depths 8 and 9 via pushed-through constant dots.
         # For any constant u: w_9 . u = w_8 . (L@u) + t_8 * (w_8 . ((R-L)@u)),
         # so sdot_9 and both output dots need only 6 matvec variants on w_8
         # (packed 4 + 2 per PSUM bank at partition offsets 0/32/64/96) and a
         # short per-point combine. w_9 is never materialized: no broadcasts,
         # no vv, no copies for the last two depths.
         for g in range(2):
             if g == 0:
                 row_math(2 * DEPTH - 3)  # t_8 for group 1
             w8 = w_bufs[0]
             f1 = ps_sdot.tile([M, F], F32, tag="sdot", name=f"fold1_{g}")
             f2 = ps_sdot.tile([M, F], F32, tag="sdot", name=f"fold2_{g}")
             for c in range(G):
                 ci = g * G + c
                 rhs = w8[:, ci * F : (ci + 1) * F]
                 nc.tensor.matmul(
                     f1, lhsT=f1E[:, c * M : (c + 1) * M], rhs=rhs,
                     start=(c == 0), stop=(c == G - 1),
                 )
                 nc.tensor.matmul(
                     f2, lhsT=f2E[:, c * M : (c + 1) * M], rhs=rhs,
                     start=(c == 0), stop=(c == G - 1),
                 )
             t8 = ttile[g]
             m1 = scratch.tile([G, F], F32, tag="m1", name=f"m1{g}")
             nc.vector.tensor_mul(m1, t8, f1[32 : 32 + G, :])
             sd9 = scratch.tile([G, F], F32, tag="sd9", name=f"sd9{g}")
             nc.vector.tensor_add(sd9, m1, f1[0:G, :])
             gt9 = scratch.tile([G, F], F32, tag="gt9", name=f"gt9{g}")
             nc.vector.tensor_mul(gt9, sd9, dd[g])
             xms9 = scratch.tile([G, F], F32, tag="xms9", name=f"xms9{g}")
             nc.gpsimd.tensor_sub(xms9, xml[g], gt9)
             t9 = tpool.tile([G, F], BF16, tag="t", name=f"t9{g}")
             nc.scalar.activation(t9, xms9, AFT.Sigmoid, scale=INV_SMOOTH)
             m2 = scratch.tile([G, F], F32, tag="m2", name=f"m2{g}")
             nc.vector.tensor_mul(m2, t8, f1[96 : 96 + G, :])
             ya = scratch.tile([G, F], F32, tag="ya", name=f"ya{g}")
             nc.vector.tensor_add(ya, m2, f1[64 : 64 + G, :])
             m3 = scratch.tile([G, F], F32, tag="m3", name=f"m3{g}")
             nc.vector.tensor_mul(m3, t8, f2[32 : 32 + G, :])
             yb = scratch.tile([G, F], F32, tag="yb", name=f"yb{g}")
             nc.vector.tensor_add(yb, m3, f2[0:G, :])
             m4 = scratch.tile([G, F], F32, tag="m4", name=f"m4{g}")
             nc.vector.tensor_mul(m4, t9, yb)
             ysb = scratch.tile([G, F], F32, tag="ysb", name=f"ysb{g}")
             nc.vector.tensor_add(ysb, m4, ya)
             nc.sync.dma_start(
                 out=y_out[g * half : (g + 1) * half].rearrange("(p f) -> p f", f=F),
                 in_=ysb,
             )

        if bench_reps > 1:
            with tc.For_i(0, bench_reps, 1):
                body()
        else:
            body()

    return nc


_CACHE = {}


def build_bench(reps):
    """Fresh module with the whole computation repeated `reps` times on-device."""
    nc = bacc.Bacc("TRN2", target_bir_lowering=False)
    _emit(nc, bench_reps=reps)
    nc.compile()
    return nc


def build_bass(compiled=True):
    """Build (and by default finalize) the Bacc module.

    compiled=False returns the pre-compile module for CoreSim runs.
    """
    if "nc" not in _CACHE:
        nc = bacc.Bacc("TRN2", target_bir_lowering=False)
        _emit(nc)
        _CACHE["nc"] = nc
    nc = _CACHE["nc"]
    if compiled and not _CACHE.get("compiled"):
        nc.compile()
        _CACHE["compiled"] = True
    return nc


def make_in_maps(x, split_points_param, values_param, left_matrix, right_matrix):
    x = np.ascontiguousarray(x, dtype=np.float32)
    shards = x.reshape(NCORES, NPTS)

    spp = np.asarray(split_points_param, dtype=np.float32)
    vp = np.asarray(values_param, dtype=np.float32)
    L = np.asarray(left_matrix, dtype=np.float32)
    R = np.asarray(right_matrix, dtype=np.float32)

    split = (1.0 / (1.0 + np.exp(-(4.0 * spp - 2.0)))).astype(np.float32)
    values = np.tile(vp * 3.0 + 1.0, M // vp.shape[0]).astype(np.float32)
    RmL = R - L
    lv = L @ values
    rv = RmL @ values
    Ls = L @ split
    Rms = RmL @ split
    A1 = L @ lv
    A2 = RmL @ lv
    B1 = L @ rv
    B2 = RmL @ rv

    splitE = np.zeros((M, G * G), NP_BF16)
    for i in range(G):
        splitE[:, i * G + i] = split
    f1E = np.zeros((M, G * M), NP_BF16)
    f2E = np.zeros((M, G * M), NP_BF16)
    for i in range(G):
        f1E[:, i * M + i] = Ls
        f1E[:, i * M + 32 + i] = Rms
        f1E[:, i * M + 64 + i] = A1
        f1E[:, i * M + 96 + i] = A2
        f2E[:, i * M + i] = B1
        f2E[:, i * M + 32 + i] = B2
    esel = np.zeros((G, G * M), NP_BF16)
    for i in range(G):
        esel[i, i * M : (i + 1) * M] = 1.0

    l0col = L[0, :].reshape(M, 1).astype(np.float32)
    rml0 = (R[0, :] - L[0, :]).reshape(M, 1).astype(np.float32)
    sp0 = np.full((G, 1), split[0], np.float32)
    b0 = np.full((G, 1), -INV_SMOOTH * split[0], np.float32)

    common = {
        "l16": L.astype(NP_BF16),
        "rml16": (R - L).astype(NP_BF16),
        "splitE": splitE,
        "f1E": f1E,
        "f2E": f2E,
        "esel": esel,
        "l0col": l0col,
        "rml0": rml0,
        "sp0": sp0,
        "b0": b0,
    }
    return [{"x": shards[i], **common} for i in range(NCORES)]


def kernel(x, split_points_param, values_param, left_matrix, right_matrix, max_depth):
    assert int(max_depth) == DEPTH
    nc = build_bass()
    in_maps = make_in_maps(
        x, split_points_param, values_param, left_matrix, right_matrix
    )
    res = run_bass_kernel_spmd(nc, in_maps, list(range(NCORES)))
    out = np.concatenate([res.results[i]["y"] for i in range(NCORES)])
    return out.astype(np.float32)
